# revision 41
# baseline (speedup 1.0000x reference)
"""DeepseekV2 MLA attention for 8 TRN2 NeuronCores (Bass/Tile), v2.

Sharding: core c handles batch b=c//4, head-group g=c%4 (4 of 16 heads).
The q_a/kv_a projections + shared kv latent / k_pe are replicated within
each batch's 4 cores (MLA's point); o_proj is row-parallel with the
4 partial outputs summed on the host during the gather step.

v2 over the baseline:
- bf16 operands everywhere (PSUM stays fp32): same PE rate as fp32r but
  half the DMA/marshal bytes, 2x DVE throughput, half the SBUF.
- every weight is streamed exactly once: both sequence halves are
  processed per stripe, instead of re-DMAing all weights per half.
- weights are pre-tiled on the host into the exact SBUF layouts, so every
  DMA is a contiguous >=1KB-per-partition transfer.
- causal mask via affine_select directly on the exp tiles (gpsimd),
  no mask constants.
- activations stay resident for the full sequence; o_proj runs once at
  the end over both halves.
"""

import numpy as np
import ml_dtypes

import concourse.bacc as bacc
import concourse.mybir as mybir
import concourse.tile as tile
from concourse.bass_utils import run_bass_kernel_spmd

F32 = mybir.dt.float32
BF16 = mybir.dt.bfloat16

# problem constants
B, S, HID, QL = 2, 1024, 2048, 1536
NH, NOPE, ROPE, VD, KVL = 16, 128, 64, 128, 512
QHD = NOPE + ROPE  # 192
EPS = 1e-6
THETA = 10000.0
HG = 4          # heads per core
GW = HG * VD    # 512, attn-cat width per core
NKT = HID // 128   # 16 contraction tiles over hidden
NQL = QL // 128    # 12
HJ = S // 2        # 512 token half

MM_DT = BF16
MM_NP = ml_dtypes.bfloat16


def _interleave_rows(w):
    # fold _interleave_perm into weight rows: out feature j = in feature perm[j]
    return np.concatenate([w[0::2], w[1::2]], axis=0)


def _rope_tables(positions):
    inv = 1.0 / (THETA ** (np.arange(0, ROPE, 2, dtype=np.float32) / ROPE))
    t = positions.astype(np.float32)
    freqs = np.outer(t, inv)
    emb = np.concatenate([freqs, freqs], axis=-1)  # [S, 64]
    return np.cos(emb), np.sin(emb)


def _rot_matrix():
    # R @ x = rotate_half(x) for 64-dim x; block-diag twice for 128 rows.
    R = np.zeros((ROPE, ROPE), np.float32)
    for j in range(32):
        R[j, j + 32] = -1.0
        R[j + 32, j] = 1.0
    R2 = np.zeros((128, 128), np.float32)
    R2[:64, :64] = R
    R2[64:, 64:] = R
    return R2


def _stripe_pack(wT, n_stripes):
    """[K, M] (contraction-major) -> [n_stripes, 128, K] stripe tiles.

    Stripe m, partition p, col k*128+kp holds wT[k*128+kp, m*128+p_free]...
    laid out so tile[:, k*128:(k+1)*128] is the [K-chunk, M-chunk] lhsT
    block for contraction chunk k of output stripe m.
    """
    K, M = wT.shape
    nk = K // 128
    assert M == n_stripes * 128
    return np.ascontiguousarray(
        wT.reshape(nk, 128, n_stripes, 128).transpose(2, 1, 0, 3).reshape(
            n_stripes, 128, nk * 128)).astype(MM_NP)


def prep_in_maps(inputs):
    """Full inputs -> list of 8 per-core input dicts (numpy, host-side)."""
    h = np.asarray(inputs["hidden_states"], np.float32)
    pos = np.asarray(inputs["position_ids"])
    q_a_w = np.asarray(inputs["q_a_w"], np.float32)
    q_a_ln = np.asarray(inputs["q_a_ln"], np.float32)
    q_b_w = np.asarray(inputs["q_b_w"], np.float32)
    kv_a_w = np.asarray(inputs["kv_a_w"], np.float32)
    kv_a_ln = np.asarray(inputs["kv_a_ln"], np.float32)
    kv_b_w = np.asarray(inputs["kv_b_w"], np.float32)
    o_w = np.asarray(inputs["o_w"], np.float32)

    wqa = _stripe_pack(np.ascontiguousarray(q_a_w.T), NQL)  # [12,128,2048]

    # kv_a: fold interleave perm into the k_pe rows (last 64), and duplicate
    # the pe block so k_pe^T materializes on both partition halves (heads at
    # partition base 0 and 64 both need an aligned copy).
    pe_rows_w = _interleave_rows(kv_a_w[KVL:])
    kv_a_w2 = np.concatenate([kv_a_w[:KVL], pe_rows_w, pe_rows_w], axis=0)  # [640, HID]
    wkva = _stripe_pack(np.ascontiguousarray(kv_a_w2.T), 5)  # [5,128,2048]

    scale = QHD ** -0.5
    rot = _rot_matrix()
    rotT = np.ascontiguousarray(rot.T).astype(MM_NP)

    per_core = []
    for c in range(8):
        b, g = divmod(c, 4)
        heads = range(HG * g, HG * g + HG)

        # q_b rows for this group, blocked [4x nope(128), 2x pe-pair(128)],
        # with q_a_ln folded into columns, interleave perm folded into pe
        # rows, and the attention scale folded in.
        nope_rows = []
        pe_rows = []
        for hh in heads:
            rows = q_b_w[hh * QHD:(hh + 1) * QHD]  # [192, QL]
            nope_rows.append(rows[:NOPE])
            pe_rows.append(_interleave_rows(rows[NOPE:]))
        wqb_g = np.concatenate(nope_rows + pe_rows, axis=0)  # [768, QL]
        wqb_g = wqb_g * q_a_ln[None, :] * scale
        wqb = _stripe_pack(np.ascontiguousarray(wqb_g.T), 6)  # [6,128,1536]

        # kv_b nope/v for this group with kv_a_ln folded
        kn_rows = []
        v_rows = []
        for hh in heads:
            rows = kv_b_w[hh * (NOPE + VD):(hh + 1) * (NOPE + VD)]
            kn_rows.append(rows[:NOPE])
            v_rows.append(rows[NOPE:])
        wkbn = np.concatenate(kn_rows, axis=0) * kv_a_ln[None, :]  # [512, KVL]
        wkbv = np.concatenate(v_rows, axis=0) * kv_a_ln[None, :]
        # [4, 128, 512]: chunk k4 is the [K-chunk, 512-out] rhs block
        wkbn = np.ascontiguousarray(wkbn.T).astype(MM_NP).reshape(4, 128, GW)
        wkbv = np.ascontiguousarray(wkbv.T).astype(MM_NP).reshape(4, 128, GW)

        # o_w columns for this group's heads (rows of o_w.T)
        woT = np.ascontiguousarray(o_w[:, GW * g: GW * (g + 1)].T)  # [512, HID]
        wo = _stripe_pack(woT, NKT)  # [16, 128, 512]

        cos, sin = _rope_tables(np.asarray(pos[b]))
        cosT = np.ascontiguousarray(np.concatenate([cos.T, cos.T], axis=0))  # [128, S]
        sinT = np.ascontiguousarray(np.concatenate([sin.T, sin.T], axis=0))

        hT = np.ascontiguousarray(h[b].T).astype(MM_NP).reshape(NKT, 128, S)

        per_core.append({
            "hT": hT,
            "wqa": wqa, "wqb": wqb, "wkva": wkva,
            "wkbn": wkbn, "wkbv": wkbv, "wo": wo,
            "cosT": cosT.astype(MM_NP), "sinT": sinT.astype(MM_NP),
            "rotT": rotT,
        })
    return per_core


def combine_outputs(results):
    """8 per-core outT [16,128,S] partials -> full [B, S, HID] output."""
    out = np.zeros((B, S, HID), np.float32)
    for c, r in enumerate(results):
        b = c // 4
        arr = np.asarray(r["outT"]).astype(np.float32).reshape(HID, S)
        out[b] += arr.T
    return out


def build_nc(debug=False):
    nc = bacc.Bacc("TRN2", target_bir_lowering=False, debug=False, num_devices=8)
    dram = nc.declare_dram_parameter

    hT = dram("hT", [NKT, 128, S], MM_DT, isOutput=False)
    wqa = dram("wqa", [NQL, 128, NKT * 128], MM_DT, isOutput=False)
    wqb = dram("wqb", [6, 128, NQL * 128], MM_DT, isOutput=False)
    wkva = dram("wkva", [5, 128, NKT * 128], MM_DT, isOutput=False)
    wkbn = dram("wkbn", [4, 128, GW], MM_DT, isOutput=False)
    wkbv = dram("wkbv", [4, 128, GW], MM_DT, isOutput=False)
    wo = dram("wo", [NKT, 128, GW], MM_DT, isOutput=False)
    cosT = dram("cosT", [128, S], MM_DT, isOutput=False)
    sinT = dram("sinT", [128, S], MM_DT, isOutput=False)
    rotT = dram("rotT", [128, 128], MM_DT, isOutput=False)
    outT = dram("outT", [NKT, 128, S], MM_DT, isOutput=True)

    AF = mybir.ActivationFunctionType
    MULT = mybir.AluOpType.mult
    ADD = mybir.AluOpType.add

    with tile.TileContext(nc) as tc:
        with (
            tc.tile_pool(name="consts", bufs=1) as consts,
            tc.tile_pool(name="sb", bufs=1) as sb,
            tc.tile_pool(name="ps", space="PSUM", bufs=1) as ps,
        ):
            # ---- constants ----
            ones_f = consts.tile([128, 1], F32, name="ones_f")
            nc.vector.memset(ones_f[:], 1.0)
            ones_r = consts.tile([128, 1], MM_DT, name="ones_r")
            nc.vector.tensor_copy(ones_r[:], ones_f[:])
            eps_sb = consts.tile([128, 1], F32, name="eps_sb")
            nc.vector.memset(eps_sb[:], EPS)
            rot_sb = consts.tile([128, 128], MM_DT, name="rot_sb")
            cos_sb = consts.tile([128, S], MM_DT, name="cos_sb")
            sin_sb = consts.tile([128, S], MM_DT, name="sin_sb")

            # ---- persistent tensors (full S) ----
            ht = [sb.tile([128, S], MM_DT, name=f"ht{k}", tag="ht", bufs=NKT)
                  for k in range(NKT)]
            ql = [sb.tile([128, S], MM_DT, name=f"ql{m}", tag="ql", bufs=NQL)
                  for m in range(NQL)]
            latn = [sb.tile([128, S], MM_DT, name=f"latn{m}", tag="latn", bufs=4)
                    for m in range(4)]
            kpeT = sb.tile([128, S], MM_DT, name="kpeT", tag="kpeT", bufs=1)
            kT = [sb.tile([128, S], MM_DT, name=f"kT{hh}", tag="kT", bufs=4)
                  for hh in range(HG)]
            vsb = [sb.tile([128, GW], MM_DT, name=f"v{i}", tag="v", bufs=8)
                   for i in range(8)]
            qT = [sb.tile([128, S], MM_DT, name=f"qT{m}", tag="qT", bufs=6)
                  for m in range(6)]
            at = [[sb.tile([128, HJ], MM_DT, name=f"at{j}_{hh}", tag="at", bufs=8)
                   for hh in range(HG)] for j in range(2)]
            Rq = [sb.tile([128, HJ], F32, name=f"Rq{j}", tag="Rq", bufs=2)
                  for j in range(2)]

            jsls = [slice(0, HJ), slice(HJ, S)]

            # ---- DMA issue order drives arrival order (the cost model
            # serializes DMA): first S3 stripe, then hidden (first-half
            # columns first so stripe-0/j=0 compute can finish early),
            # then the small tables/weights needed later.
            wkva_sb = [sb.tile([128, NKT * 128], MM_DT, name=f"wkva{m}", tag="wkva", bufs=2)
                       for m in range(5)]
            nc.sync.dma_start(wkva_sb[0][:], wkva[0, :, :])
            for k in range(NKT):
                nc.sync.dma_start(ht[k][:], hT[k, :, :])
            nc.sync.dma_start(wkva_sb[1][:], wkva[1, :, :])
            nc.sync.dma_start(cos_sb[:], cosT[:, :])
            nc.sync.dma_start(sin_sb[:], sinT[:, :])
            nc.sync.dma_start(rot_sb[:], rotT[:, :])
            kbn_sb = []
            kbv_sb = []
            for k4 in range(4):
                tn = sb.tile([128, GW], MM_DT, name=f"kbn{k4}", tag="kbn", bufs=4)
                nc.sync.dma_start(tn[:], wkbn[k4, :, :])
                kbn_sb.append(tn)
                tv = sb.tile([128, GW], MM_DT, name=f"kbv{k4}", tag="kbv", bufs=4)
                nc.sync.dma_start(tv[:], wkbv[k4, :, :])
                kbv_sb.append(tv)

            def rope_combine(dst, pm_src, xq, j):
                """dst = interleaved(src)*cos + rotate_half(interleaved(src))*sin.

                xq is a scratch bf16 copy of the pe pre-activation; the perm
                is folded into the weights already.
                """
                jsl = jsls[j]
                pr = ps.tile([128, HJ], F32, name=f"ps_rot_{id(dst)}_{j}", tag="mm", bufs=4)
                nc.tensor.matmul(pr[:], rot_sb[:], xq[:], start=True, stop=True)
                # t1 on the idle gpsimd engine, t2 (PSUM-reading) on DVE:
                # the rope chain no longer serializes on one engine.
                t1 = sb.tile([128, HJ], F32, name=f"t1_{id(dst)}_{j}", tag="tmp", bufs=4)
                nc.gpsimd.tensor_tensor(out=t1[:], in0=xq[:], in1=cos_sb[:, jsl], op=MULT)
                t2 = sb.tile([128, HJ], F32, name=f"t2_{id(dst)}_{j}", tag="tmp", bufs=4)
                nc.vector.tensor_tensor(out=t2[:], in0=pr[:], in1=sin_sb[:, jsl], op=MULT)
                return t1, t2

            # ---- S3: kv_a -> latent (4 stripes) + k_pe stripe ----
            ps_msk = [ps.tile([1, HJ], F32, name=f"msq_kv{j}", tag="row", bufs=2)
                      for j in range(2)]
            for m in range(5):
                wst = wkva_sb[m]
                if m > 1:
                    nc.sync.dma_start(wst[:], wkva[m, :, :])
                for j in range(2):
                    jsl = jsls[j]
                    pm = ps.tile([128, HJ], F32, name=f"ps_kva{m}_{j}", tag="mm", bufs=4)
                    for k in range(NKT):
                        nc.tensor.matmul(pm[:], wst[:, k * 128:(k + 1) * 128],
                                         ht[k][:, jsl], start=(k == 0), stop=(k == NKT - 1))
                    if m < 4:
                        nc.vector.tensor_copy(latn[m][:, jsl], pm[:])
                        sqt = sb.tile([128, HJ], MM_DT, name=f"sq_kv{m}_{j}", tag="tmp2", bufs=4)
                        nc.scalar.activation(sqt[:], pm[:], AF.Square)
                        nc.tensor.matmul(ps_msk[j][:], ones_r[:], sqt[:],
                                         start=(m == 0), stop=(m == 3))
                    else:
                        xpe = sb.tile([128, HJ], MM_DT, name=f"xpe{j}", tag="tmp2", bufs=4)
                        nc.vector.tensor_copy(xpe[:], pm[:])
                        t1, t2 = rope_combine(kpeT, pm, xpe, j)
                        nc.vector.tensor_tensor(out=kpeT[:, jsl], in0=t1[:], in1=t2[:], op=ADD)

            # kv rmsnorm scale rows; normalize latent in place
            for j in range(2):
                jsl = jsls[j]
                sr = sb.tile([1, HJ], F32, name=f"sr_k{j}", tag="srow", bufs=4)
                nc.scalar.activation(sr[:], ps_msk[j][:], AF.Sqrt, bias=eps_sb[0:1, :],
                                     scale=1.0 / KVL)
                rr = sb.tile([1, HJ], F32, name=f"rr_k{j}", tag="srow", bufs=4)
                nc.vector.reciprocal(rr[:], sr[:])
                Rkv = sb.tile([128, HJ], F32, name=f"Rkv{j}", tag="bcast", bufs=4)
                nc.gpsimd.partition_broadcast(Rkv[:], rr[:])
                for m in range(4):
                    nc.vector.tensor_tensor(out=latn[m][:, jsl], in0=latn[m][:, jsl],
                                            in1=Rkv[:], op=MULT)

            # ---- S1 (first two stripes, to cover the kv-norm latency) ----
            ps_msq = [ps.tile([1, HJ], F32, name=f"msq_q{j}", tag="row", bufs=2)
                      for j in range(2)]

            def s1_stripe(m):
                wst = sb.tile([128, NKT * 128], MM_DT, name=f"wqa{m}", tag="wqa", bufs=3)
                nc.sync.dma_start(wst[:], wqa[m, :, :])
                for j in range(2):
                    jsl = jsls[j]
                    pm = ps.tile([128, HJ], F32, name=f"ps_qa{m}_{j}", tag="mm", bufs=4)
                    for k in range(NKT):
                        nc.tensor.matmul(pm[:], wst[:, k * 128:(k + 1) * 128],
                                         ht[k][:, jsl], start=(k == 0), stop=(k == NKT - 1))
                    nc.vector.tensor_copy(ql[m][:, jsl], pm[:])
                    sqt = sb.tile([128, HJ], MM_DT, name=f"sq_q{m}_{j}", tag="tmp2", bufs=4)
                    nc.scalar.activation(sqt[:], pm[:], AF.Square)
                    nc.tensor.matmul(ps_msq[j][:], ones_r[:], sqt[:],
                                     start=(m == 0), stop=(m == NQL - 1))

            s1_stripe(0)
            s1_stripe(1)

            # ---- S4: k_nope^T per head ----
            for j in range(2):
                jsl = jsls[j]
                for hh in range(HG):
                    pm = ps.tile([128, HJ], F32, name=f"ps_kn{j}_{hh}", tag="mm", bufs=4)
                    for k4 in range(4):
                        nc.tensor.matmul(pm[:], kbn_sb[k4][:, hh * 128:(hh + 1) * 128],
                                         latn[k4][:, jsl], start=(k4 == 0), stop=(k4 == 3))
                    nc.scalar.copy(kT[hh][:, jsl], pm[:])

            # ---- S5: v (natural layout) per 128-token chunk ----
            for i in range(8):
                csl = slice(i * 128, (i + 1) * 128)
                pm = ps.tile([128, GW], F32, name=f"ps_v{i}", tag="mm", bufs=4)
                for k4 in range(4):
                    nc.tensor.matmul(pm[:], latn[k4][:, csl], kbv_sb[k4][:],
                                     start=(k4 == 0), stop=(k4 == 3))
                nc.scalar.copy(vsb[i][:], pm[:])

            # ---- S1: remaining q_a stripes ----
            for m in range(2, NQL):
                s1_stripe(m)

            # q rmsnorm scale rows
            for j in range(2):
                sr = sb.tile([1, HJ], F32, name=f"sr_q{j}", tag="srow", bufs=4)
                nc.scalar.activation(sr[:], ps_msq[j][:], AF.Sqrt, bias=eps_sb[0:1, :],
                                     scale=1.0 / QL)
                rr = sb.tile([1, HJ], F32, name=f"rr_q{j}", tag="srow", bufs=4)
                nc.vector.reciprocal(rr[:], sr[:])
                nc.gpsimd.partition_broadcast(Rq[j][:], rr[:])

            # ---- S2: q^T stripes; pe pairs first, then per-head + attention ----
            def s2_stripe(m):
                wst = sb.tile([128, NQL * 128], MM_DT, name=f"wqb{m}", tag="wqb", bufs=2)
                nc.sync.dma_start(wst[:], wqb[m, :, :])
                for j in range(2):
                    jsl = jsls[j]
                    pm = ps.tile([128, HJ], F32, name=f"ps_qb{m}_{j}", tag="mm", bufs=4)
                    for k in range(NQL):
                        nc.tensor.matmul(pm[:], wst[:, k * 128:(k + 1) * 128],
                                         ql[k][:, jsl], start=(k == 0), stop=(k == NQL - 1))
                    if m < 4:
                        nc.vector.tensor_tensor(out=qT[m][:, jsl], in0=pm[:],
                                                in1=Rq[j][:], op=MULT)
                    else:
                        xq = sb.tile([128, HJ], MM_DT, name=f"xq{m}_{j}", tag="tmp2", bufs=4)
                        nc.scalar.copy(xq[:], pm[:])
                        t1, t2 = rope_combine(qT[m], pm, xq, j)
                        t3 = sb.tile([128, HJ], F32, name=f"t3_{m}_{j}", tag="tmp", bufs=4)
                        nc.vector.tensor_tensor(out=t3[:], in0=t1[:], in1=t2[:], op=ADD)
                        nc.gpsimd.tensor_tensor(out=qT[m][:, jsl], in0=t3[:],
                                                in1=Rq[j][:], op=MULT)

            s2_stripe(4)
            s2_stripe(5)

            def attention(j, hh):
                # Scores are emitted a few i-tiles ahead of the sum/po
                # accumulations so the exp/mask latency never bubbles the
                # in-order PE queue.
                jsl = jsls[j]
                pe0 = (hh % 2) * 64
                qpe = qT[4 + hh // 2][pe0:pe0 + 64, jsl]
                po = ps.tile([128, HJ], F32, name=f"ps_o{j}_{hh}", tag="acc", bufs=2)
                psum = ps.tile([1, HJ], F32, name=f"ps_sum{j}_{hh}", tag="row", bufs=2)
                n = 4 * (j + 1)
                ets = [None] * n

                def emit_scores(i):
                    pss = ps.tile([128, HJ], F32, name=f"ps_s{j}_{hh}_{i}", tag="mm", bufs=4)
                    nc.tensor.matmul(pss[:], kT[hh][:, i * 128:(i + 1) * 128],
                                     qT[hh][:, jsl], start=True, stop=False)
                    nc.tensor.matmul(pss[:], kpeT[pe0:pe0 + 64, i * 128:(i + 1) * 128],
                                     qpe, start=False, stop=True)
                    et = sb.tile([128, HJ], MM_DT, name=f"e{j}_{hh}_{i}", tag="expT", bufs=6)
                    nc.scalar.activation(et[:], pss[:], AF.Exp)
                    if i >= 4 * j:  # diagonal-crossing tile: causal mask
                        nc.gpsimd.affine_select(
                            out=et[:], in_=et[:],
                            compare_op=mybir.AluOpType.is_ge, fill=0.0,
                            base=-128 * (i - 4 * j), pattern=[[1, HJ]],
                            channel_multiplier=-1)
                    ets[i] = et

                # Masked (diagonal) tiles are emitted first: the accumulation
                # is order-independent, and the tail accums then wait only on
                # exp, never on the gpsimd mask chain.
                order = list(range(4 * j, n)) + list(range(0, 4 * j))

                def emit_accum(pos):
                    i = order[pos]
                    nc.tensor.matmul(psum[:], ones_r[:], ets[i][:],
                                     start=(pos == 0), stop=(pos == n - 1))
                    nc.tensor.matmul(po[:], vsb[i][:, hh * 128:(hh + 1) * 128], ets[i][:],
                                     start=(pos == 0), stop=(pos == n - 1))

                LOOKAHEAD = 3
                for pos in range(n):
                    emit_scores(order[pos])
                    if pos >= LOOKAHEAD:
                        emit_accum(pos - LOOKAHEAD)
                for pos in range(max(0, n - LOOKAHEAD), n):
                    emit_accum(pos)

                rs = sb.tile([1, HJ], F32, name=f"rs{j}_{hh}", tag="srow", bufs=4)
                nc.vector.reciprocal(rs[:], psum[:])
                Rs = sb.tile([128, HJ], F32, name=f"Rs{j}_{hh}", tag="bcast", bufs=4)
                nc.gpsimd.partition_broadcast(Rs[:], rs[:])
                nc.vector.tensor_tensor(out=at[j][hh][:], in0=po[:], in1=Rs[:], op=MULT)

            for hh in range(HG):
                s2_stripe(hh)
                attention(0, hh)

            # ---- o_proj transposed: outT[o, t] = sum_c wo[c, o] attn_catT[c, t].
            # wo stays resident so the j=0 half can drain while the j=1
            # attention still runs.
            wos_sb = [sb.tile([128, GW], MM_DT, name=f"wo{oc}", tag="wos", bufs=NKT)
                      for oc in range(NKT)]

            def o_proj(oc, j):
                jsl = jsls[j]
                pm = ps.tile([128, HJ], F32, name=f"ps_out{oc}_{j}", tag="mm", bufs=4)
                for hh in range(HG):
                    nc.tensor.matmul(pm[:], wos_sb[oc][:, hh * 128:(hh + 1) * 128],
                                     at[j][hh][:], start=(hh == 0), stop=(hh == HG - 1))
                ot = sb.tile([128, HJ], MM_DT, name=f"ot{oc}_{j}", tag="osb", bufs=4)
                nc.vector.tensor_copy(ot[:], pm[:])
                nc.sync.dma_start(outT[oc, :, jsl], ot[:])

            attention(1, 0)
            attention(1, 1)
            for oc in range(NKT):
                nc.sync.dma_start(wos_sb[oc][:], wo[oc, :, :])
                o_proj(oc, 0)
            attention(1, 2)
            attention(1, 3)
            for oc in range(NKT):
                o_proj(oc, 1)

    nc.compile()
    return nc


_NC = None


def _get_nc():
    global _NC
    if _NC is None:
        _NC = build_nc()
    return _NC


def run(inputs, trace=False):
    in_maps = prep_in_maps(inputs)
    nc = _get_nc()
    res = run_bass_kernel_spmd(nc, in_maps, core_ids=list(range(8)), trace=trace)
    out = combine_outputs(res.results)
    return out, res


def kernel(**inputs):
    out, _ = run(inputs)
    return out.astype(np.float32)
